# revision 10
# baseline (speedup 1.0000x reference)
"""Multi-head attention (S=2048, B=2, D=1024, H=16) on 8 trn2 NeuronCores.

Sharding: 2 heads per core (head parallelism). Each core computes Q/K/V
projections for its 128 output features, attention for its 4 (batch,
head) pairs, and a partial output projection; the host sums the 8
partial outputs.

v2 restructure vs the 261us baseline (which idled the Scalar/exp engine
107us): the exp stream is the kernel's hard floor (134M exps / core /
153.6G/s ~= 110us + per-instruction overhead = 147us), so everything is
scheduled around keeping it continuous:
 - batch-major token layout ([D, B*S]) kills the strided deinterleave
   copies and lets attention chunks depend on one batch's K/V only.
 - b-outer chunk order (b0 qc0-3 then b1 qc0-3) with a hand-ordered DMA
   priority stream: k(b0) + q(b0,0) first, so exp starts ~10us in
   instead of 30us, and b1's inputs stream under b0's attention.
 - per-slot emission order [exp(g) | scores(g+2) | folds | av(g)]
   keeps the scores that gate the NEXT exp ahead of all other PE work
   in the engine FIFO (scores lead the exp stream by 2 key-tiles,
   crossing chunk boundaries), so av/proj/oproj stalls never block it.
 - q/k/v projections for later chunks, v transposes, and the previous
   chunks' output projections are folded into explicit slots of the
   attention stream, each sized under the ~290ns/slot PE slack.
 - kT/qT/xT/wo are fp16 (half DMA + SBUF, FWL weight loads); scores /
   exp / attention accumulate stay fp32/f32r.  Partial outputs ship
   fp16 (halves the 16MB output traffic; host sums in fp32).
"""
import sys
sys.path.insert(0, '/opt/trn_rl_repo')
import functools
import os

import numpy as np

import concourse.bacc as bacc
import concourse.mybir as mybir
import concourse.tile as tile
from concourse.bass_utils import run_bass_kernel_spmd
from concourse.masks import make_identity

F32 = mybir.dt.float32
F32R = mybir.dt.float32r
F16 = mybir.dt.float16
BF16 = mybir.dt.bfloat16
AFT = mybir.ActivationFunctionType
MUL = mybir.AluOpType.mult

S, B, D, H = 2048, 2, 1024, 16
T = S * B               # 4096 tokens
DK = D // H             # 64
NC = 8                  # cores
FPC = D // NC           # 128 features per core (2 heads)
QC = 512                # q-chunk size
JT = S // 128           # 16 key tiles per batch
DT = D // 128           # 8 contraction tiles for projections
NCH = 8                 # chunks, b-outer: c -> b=c//4, qc=c%4
NSLOT = NCH * JT        # 128 global (chunk, key-tile) slots

EJ_DT = {"f32r": F32R, "bf16": BF16}[os.environ.get("EJ_DT", "f32r")]
OUT_DT = {"fp16": F16, "fp32": F32}[os.environ.get("OUT_DT", "fp16")]


def build_nc():
    nc = bacc.Bacc(None, target_bir_lowering=False)

    xq = nc.dram_tensor("xq", [D, T], F16, kind="ExternalInput")
    xk = nc.dram_tensor("xk", [D, T], F16, kind="ExternalInput")
    xv = nc.dram_tensor("xv", [D, T], F16, kind="ExternalInput")
    wq = nc.dram_tensor("wq", [D, FPC], F16, kind="ExternalInput")
    wk = nc.dram_tensor("wk", [D, FPC], F16, kind="ExternalInput")
    wv = nc.dram_tensor("wv", [D, FPC], F16, kind="ExternalInput")
    wo = nc.dram_tensor("wo", [FPC, D], F16, kind="ExternalInput")
    out = nc.dram_tensor("out", [T, D], OUT_DT, kind="ExternalOutput")
    DBG = os.environ.get("DEBUG_DUMP", "0") == "1"
    if DBG:
        dbg_q = nc.dram_tensor("dbg_q", [128, S], F16, kind="ExternalOutput")
        dbg_k = nc.dram_tensor("dbg_k", [128, S], F16, kind="ExternalOutput")
        dbg_vb = nc.dram_tensor("dbg_vb", [128, JT * 130], F32, kind="ExternalOutput")
        dbg_den = nc.dram_tensor("dbg_den", [2, QC], F32, kind="ExternalOutput")
        dbg_ej = nc.dram_tensor("dbg_ej", [128, 2 * QC], F32, kind="ExternalOutput")
        dbg_ej2 = nc.dram_tensor("dbg_ej2", [128, 2 * QC], F32, kind="ExternalOutput")
        dbg_p0 = nc.dram_tensor("dbg_p0", [65, QC], F32, kind="ExternalOutput")
        dbg_p1 = nc.dram_tensor("dbg_p1", [65, QC], F32, kind="ExternalOutput")
    xsrc = {"q": xq, "k": xk, "v": xv}

    with tile.TileContext(nc) as tc:
        with (
            tc.tile_pool(name="wpool", bufs=1) as wpool,
            tc.tile_pool(name="proj", bufs=1) as projpool,
            tc.tile_pool(name="vtmp", bufs=2) as vtpool,
            tc.tile_pool(name="xdma", bufs=6) as xpool,
            tc.tile_pool(name="ej", bufs=8) as epool,
            tc.tile_pool(name="norm", bufs=2) as npool,
            tc.tile_pool(name="osb", bufs=3) as opool,
            tc.tile_pool(name="psS", bufs=2, space="PSUM") as psS,
            tc.tile_pool(name="psA", bufs=2, space="PSUM") as psA,
            tc.tile_pool(name="psM", bufs=2, space="PSUM") as psM,
        ):
            # ---- weights / constants ----
            w_t = {}
            for name, wd in (("k", wk), ("q", wq), ("v", wv)):
                w_t[name] = wpool.tile([128, DT, FPC], F16, name=f"w_{name}")
                nc.sync.dma_start(w_t[name][:], wd.rearrange("(t p) m -> p t m", p=128))
            ident = wpool.tile([128, 128], F32, name="ident")
            make_identity(nc, ident[:])
            wo_t = wpool.tile([128, D], F16, name="wo_t")

            # ---- persistent activations ----
            kT = [projpool.tile([128, S], F16, name=f"kT{b}") for b in range(B)]
            qT = [projpool.tile([128, S], F16, name=f"qT{b}") for b in range(B)]
            v_b = [projpool.tile([128, JT, 130], F32R, name=f"v_b{b}") for b in range(B)]
            xT = projpool.tile([128, T], F16, name="xT")
            # ones columns of v_b (denominator trick) are static
            for b in range(B):
                for jt in range(JT):
                    nc.vector.memset(v_b[b][:, jt, 64:65].bitcast(F32), 1.0)
                    nc.vector.memset(v_b[b][:, jt, 129:130].bitcast(F32), 1.0)

            # ---- input DMA stream (emission order == transfer order) ----
            xt_tiles = {}

            def emit_xdma(p, b, hf):
                t = xpool.tile([128, DT, 512], F16, name="xt", tag="xt")
                tok0 = b * S + hf * 512
                nc.sync.dma_start(
                    t[:],
                    xsrc[p].rearrange("(a p) m -> p a m", p=128)[:, :, tok0:tok0 + 512])
                xt_tiles[(p, b, hf)] = t

            dma_order = [
                ("k", 0, 0), ("q", 0, 0), ("k", 0, 1), ("v", 0, 0), ("k", 0, 2),
                ("k", 0, 3), ("v", 0, 1), ("q", 0, 1), ("v", 0, 2),
                "WO",
                ("v", 0, 3), ("q", 0, 2), ("k", 1, 0), ("k", 1, 1), ("k", 1, 2),
                ("k", 1, 3), ("q", 0, 3), ("v", 1, 0), ("v", 1, 1), ("v", 1, 2),
                ("v", 1, 3), ("q", 1, 0), ("q", 1, 1), ("q", 1, 2), ("q", 1, 3),
            ]
            for item in dma_order:
                if item == "WO":
                    nc.sync.dma_start(wo_t[:], wo[:, :])
                else:
                    emit_xdma(*item)

            # ---- building blocks ----
            def proj_mms(p, b, hf, dest):
                """8 accumulating matmuls + 1 evac copy for 512 tokens."""
                xt = xt_tiles.pop((p, b, hf))
                ps = psM.tile([128, 512], F32, name="psproj", tag="m")
                for dt in range(DT):
                    nc.tensor.matmul(ps[:], w_t[p][:, dt, :], xt[:, dt, :],
                                     start=(dt == 0), stop=(dt == DT - 1))
                nc.vector.tensor_copy(dest, ps[:])

            vstore = {}

            def v_proj(b, hf):
                vt = vtpool.tile([128, 512], F32R, name="vt", tag="vt")
                proj_mms("v", b, hf, vt[:])
                vstore[(b, hf)] = vt

            def v_tp(b, hf, jj):
                """transpose 2 of the 4 key-tiles of v half hf into v_b."""
                vt = vstore[(b, hf)]
                for u in range(2):
                    q = 2 * jj + u
                    jt = 4 * hf + q
                    tp = psM.tile([128, 128], F32, name="tp", tag="m")
                    nc.tensor.transpose(
                        tp[:], vt[:, q * 128:(q + 1) * 128].bitcast(F32), ident[:])
                    nc.vector.tensor_copy(v_b[b][:, jt, 0:64], tp[:, 0:64])
                    nc.vector.tensor_copy(v_b[b][:, jt, 65:129], tp[:, 64:128])

            sj_t = {}
            ej_t = {}

            def emit_scores(g):
                c, j = divmod(g, JT)
                b, qc = divmod(c, 4)
                sj = psS.tile([128, 2, QC], F32, name="sj", tag="sj")
                for h in range(2):
                    nc.tensor.matmul(
                        sj[:, h, :], kT[b][h * 64:(h + 1) * 64, j * 128:(j + 1) * 128],
                        qT[b][h * 64:(h + 1) * 64, qc * QC:(qc + 1) * QC],
                        start=True, stop=True)
                sj_t[g] = sj

            def emit_exp(g):
                ej = epool.tile([128, 2, QC], EJ_DT, name="ej", tag="ej")
                nc.scalar.activation(ej[:], sj_t.pop(g)[:], AFT.Exp)
                if DBG and g in (0, 15):
                    ejd = npool.tile([128, 2 * QC], F32, name="ejd", tag="ejd")
                    nc.vector.tensor_copy(ejd[:], ej[:].rearrange("p a m -> p (a m)"))
                    nc.sync.dma_start((dbg_ej if g == 0 else dbg_ej2)[:, :], ejd[:])
                ej_t[g] = ej

            def emit_av(g, pacc):
                c, j = divmod(g, JT)
                b = c // 4
                ej = ej_t.pop(g)
                for h in range(2):
                    nc.tensor.matmul(
                        pacc[h][0:65, :], v_b[b][:, j, h * 65:h * 65 + 65],
                        ej[:, h, :], start=(j == 0), stop=(j == JT - 1))

            def emit_normalize(c, pacc):
                if DBG and c == 0:
                    for h, dst in ((0, dbg_p0), (1, dbg_p1)):
                        pd = npool.tile([65, QC], F32, name="pd", tag="pd")
                        nc.vector.tensor_copy(pd[:], pacc[h][0:65, :])
                        nc.sync.dma_start(dst[:, :], pd[:])
                for h in range(2):
                    dsb = npool.tile([1, QC], F32, name="dsb", tag="dsb")
                    nc.vector.tensor_copy(dsb[:], pacc[h][64:65, :])
                    if DBG and c == 0:
                        nc.sync.dma_start(dbg_den[h:h + 1, :], dsb[:])
                    rd = npool.tile([1, QC], F32, name="rd", tag="rd")
                    nc.vector.reciprocal_approx_fast(out=rd[:], in_=dsb[:])
                    bc = npool.tile([64, QC], F32, name="bc", tag="bc")
                    nc.gpsimd.partition_broadcast(bc[:], rd[:])
                    nc.vector.tensor_tensor(
                        out=xT[h * 64:(h + 1) * 64, c * QC:(c + 1) * QC],
                        in0=pacc[h][0:64, :], in1=bc[:], op=MUL)

            def emit_oproj(tt, use_act=False):
                osb = opool.tile([128, D], OUT_DT, name="osb", tag="osb")
                for ec in range(2):
                    po = psM.tile([128, 512], F32, name="po", tag="m")
                    nc.tensor.matmul(po[:], xT[:, tt * 128:(tt + 1) * 128],
                                     wo_t[:, ec * 512:(ec + 1) * 512],
                                     start=True, stop=True)
                    if use_act and ec == 0:
                        nc.scalar.copy(osb[:, 0:512], po[:])
                    else:
                        nc.vector.tensor_copy(osb[:, ec * 512:(ec + 1) * 512], po[:])
                nc.gpsimd.dma_start(out[tt * 128:(tt + 1) * 128, :], osb[:])

            # ---- fold schedule: slot -> thunks (sized ~<=1.7us PE each) ----
            def k_item(b, hf):
                return lambda: proj_mms("k", b, hf, kT[b][:, hf * 512:(hf + 1) * 512])

            def q_item(c):
                b, qc = divmod(c, 4)
                return lambda: proj_mms("q", b, qc, qT[b][:, qc * 512:(qc + 1) * 512])

            def vp_item(b, hf):
                return lambda: v_proj(b, hf)

            def vt_item(b, hf, jj):
                return lambda: v_tp(b, hf, jj)

            def op_item(tt):
                return lambda: emit_oproj(tt)

            FOLD = {}

            def put(g, th):
                FOLD.setdefault(g, []).append(th)

            # b0 k halves 1-3 early in chunk 0 (DMA-paced)
            put(2, k_item(0, 1)); put(5, k_item(0, 2)); put(8, k_item(0, 3))
            # b0 v halves + transposes, DMA-paced through chunk 0; the avs
            # of chunk 0 are deferred until their v_b tiles exist (AV_SLOT)
            put(4, vp_item(0, 0)); put(5, vt_item(0, 0, 0)); put(6, vt_item(0, 0, 1))
            put(9, vp_item(0, 1)); put(10, vt_item(0, 1, 0)); put(10, vt_item(0, 1, 1))
            put(13, vp_item(0, 2)); put(14, vt_item(0, 2, 0)); put(14, vt_item(0, 2, 1))
            put(17, vp_item(0, 3)); put(18, vt_item(0, 3, 0)); put(18, vt_item(0, 3, 1))
            # q projections: chunk c's q one chunk ahead (c0's q in prologue)
            put(13, q_item(1)); put(29, q_item(2)); put(45, q_item(3))
            put(60, q_item(4)); put(77, q_item(5)); put(93, q_item(6)); put(109, q_item(7))
            # b1 k/v prep under b0's chunks 2-3
            put(34, k_item(1, 0)); put(37, k_item(1, 1))
            put(40, k_item(1, 2)); put(43, k_item(1, 3))
            put(50, vp_item(1, 0)); put(51, vt_item(1, 0, 0)); put(52, vt_item(1, 0, 1))
            put(53, vp_item(1, 1)); put(54, vt_item(1, 1, 0)); put(55, vt_item(1, 1, 1))
            put(56, vp_item(1, 2)); put(57, vt_item(1, 2, 0)); put(58, vt_item(1, 2, 1))
            put(59, vp_item(1, 3)); put(61, vt_item(1, 3, 0)); put(62, vt_item(1, 3, 1))
            # output projection tiles: chunk c's 4 tiles folded 1-2 chunks later
            op_slots = {0: (25, 27, 30, 31), 1: (38, 41, 44, 46),
                        2: (70, 72, 74, 76), 3: (82, 84, 86, 88),
                        4: (98, 100, 102, 104), 5: (114, 116, 118, 120),
                        6: (121, 123, 125, 127)}
            for c, slots in op_slots.items():
                for i, g in enumerate(slots):
                    put(g, op_item(4 * c + i))

            # ---- prologue compute ----
            proj_mms("k", 0, 0, kT[0][:, 0:512])
            proj_mms("q", 0, 0, qT[0][:, 0:512])
            emit_scores(0)
            emit_scores(1)

            # ---- av emission slots: an av can only be emitted once the
            # v_tp that writes its v_b key-tile has been emitted (chunk 0
            # streams under the input DMA, so its avs defer), and a chunk's
            # avs must all precede the next chunk's (pacc bank rotation).
            AV_SLOT = {}
            prev_last = -1
            for g in range(NSLOT):
                c, j = divmod(g, JT)
                if c == 0:
                    s = 6 + 4 * (j // 4)      # v_b[0] tiles ready at 6/10/14/18
                else:
                    s = max(g, prev_last + 1)
                if j == JT - 1:
                    prev_last = s
                AV_SLOT.setdefault(s, []).append(g)

            # ---- the stream ----
            paccs = {}
            for g in range(NSLOT):
                c, j = divmod(g, JT)
                if j == 0:
                    paccs[c] = [psA.tile([128, QC], F32, name=f"pacc{h}", tag="pacc")
                                for h in range(2)]
                emit_exp(g)
                if g + 2 < NSLOT:
                    emit_scores(g + 2)
                for th in FOLD.get(g, ()):
                    th()
                for ga in AV_SLOT.get(g, ()):
                    ca = ga // JT
                    emit_av(ga, paccs[ca])
                    if ga % JT == JT - 1:
                        emit_normalize(ca, paccs.pop(ca))

            # ---- tail: last chunk's output projection ----
            for tt in range(28, 32):
                emit_oproj(tt, use_act=True)
            if DBG:
                nc.sync.dma_start(dbg_q[:, :], qT[0][:])
                nc.sync.dma_start(dbg_k[:, :], kT[0][:])
                nc.sync.dma_start(
                    dbg_vb[:, :], v_b[0][:].bitcast(F32).rearrange("p a m -> p (a m)"))
    nc.finalize()
    return nc


@functools.cache
def _nc_cached():
    return build_nc()


def _prep_in_maps(inputs):
    np16 = np.float16

    def xbm(a):
        # [S,B,D] -> [D, B*S] batch-major tokens, fp16
        a = np.asarray(a, np.float32).transpose(2, 1, 0).reshape(D, T)
        return np.ascontiguousarray(a).astype(np16)

    xq_h = xbm(inputs["query"])
    xk_h = xbm(inputs["key"])
    xv_h = xbm(inputs["value"])
    Wq, Wk, Wv, Wo = (np.asarray(inputs[k], np.float32) for k in ("Wq", "Wk", "Wv", "Wo"))

    in_maps = []
    for c in range(NC):
        sl = slice(c * FPC, (c + 1) * FPC)
        in_maps.append({
            "xq": xq_h, "xk": xk_h, "xv": xv_h,
            "wq": np.ascontiguousarray(Wq[sl, :].T).astype(np16),
            "wk": np.ascontiguousarray(Wk[sl, :].T).astype(np16),
            "wv": np.ascontiguousarray(Wv[sl, :].T).astype(np16),
            "wo": np.ascontiguousarray(Wo[:, sl].T).astype(np16),
        })
    return in_maps


def kernel(query, key, value, Wq, bq, Wk, bk, Wv, bv, Wo, bo):
    in_maps = _prep_in_maps({"query": query, "key": key, "value": value,
                             "Wq": Wq, "Wk": Wk, "Wv": Wv, "Wo": Wo})
    nc = _nc_cached()
    res = run_bass_kernel_spmd(nc, in_maps, core_ids=list(range(NC)))
    acc = np.zeros((T, D), np.float32)
    for r in res.results:
        acc += r["out"].astype(np.float32)
    acc += np.asarray(bo, np.float32)[None, :]
    # batch-major tokens back to [S, B, D]
    out = acc.reshape(B, S, D).transpose(1, 0, 2)
    out = np.ascontiguousarray(out, np.float32)
    for bias in (bq, bk, bv):
        assert float(np.abs(np.asarray(bias)).max()) == 0.0, "nonzero qkv bias unsupported"
    return out
